# revision 41
# baseline (speedup 1.0000x reference)
"""Trainium2 Bass kernel for nn_BaseViewTransform (BEVFusion bev_pool / segment-mean).

Pipeline (v3 — hybrid int8-pair / bf16-direct windows):
  The machine balance: HBM DMA wants few bytes (int8), but int8 needs an
  on-device convert to bf16 for the PE, and only the DVE can do that at
  ~1 elem/ns/partition (GPSIMD/multi-engine runs CONTEND and go slower).
  So windows alternate between two types:
    A (int8 pairs):  80 B/pt DMA + DVE pair-add (int8+int8 -> bf16, exact)
    D (bf16 direct): 160 B/pt DMA + zero DVE work (feats feed the PE
                     straight from the DMA tile)
  which equalizes the DMA wall and the DVE wall.

  Host (index plane only): compute per-point voxel ids exactly as the
  reference, sort kept points by segment, shard across 8 cores, pack into
  128-pseudo chunks (pseudo = same-segment pair in A windows, single point
  in D windows) with <= WIN=12 distinct segments per chunk; per-chunk int8
  scales for A chunks.
  Device, per 42-chunk window: DMA stream in; [A only] DVE pair-add;
  DVE one-hot via is_equal; 42 matmuls with the FEATURES as the stationary
  operand padded to 128 columns (FWL) and the one-hot moving (12 cols);
  PSUM [0:80] -> SBUF bf16 via ACT; DMA out.
  Host: scale chunk sums, reduce per segment, divide by counts, scatter.
"""

import numpy as np
import ml_dtypes

# ---------------- problem constants (hardcoded per task rules) ----------------
IMAGE_SIZE = (256, 704)
FEATURE_SIZE = (32, 88)
XBOUND = (-54.0, 54.0, 0.3)
YBOUND = (-54.0, 54.0, 0.3)
ZBOUND = (-10.0, 10.0, 20.0)
DBOUND = (1.0, 60.0, 0.5)
C_OUT = 80
NX = (360, 360, 1)
NSEG = NX[2] * NX[0] * NX[1]  # 129600
DX = np.array([XBOUND[2], YBOUND[2], ZBOUND[2]], np.float32)
BX = np.array([XBOUND[0] + XBOUND[2] / 2.0,
               YBOUND[0] + YBOUND[2] / 2.0,
               ZBOUND[0] + ZBOUND[2] / 2.0], np.float32)

NCORES = 8
P = 128          # pseudo-points per chunk (= matmul contraction dim)
WIN = 10         # max distinct segments per chunk (= one-hot width)
CPW = 51         # chunks per window (51*10 = 510 <= 512 fp32 PSUM bank)
PAD = P - C_OUT  # lhsT column padding to 128 cols so FWL triggers
WB = CPW * C_OUT  # feature elems per window block per partition (3360)
# window type pattern: 1 A window (int8 pairs) : 2 D windows (fp8 e3m4)
APAT = 3         # pattern period; w % APAT == 0 -> A window
F8MAX = 14.0     # fp8 e3m4 scaling target (max finite 15.5, margin)


def _wtype(w):
    # A window last in each period so the program starts on D windows
    # (no pair-add dependency -> matmuls start right after the first DMA)
    return 'A' if w % APAT == APAT - 1 else 'D'


def _frustum():
    iH, iW = IMAGE_SIZE
    fH, fW = FEATURE_SIZE
    ds = np.arange(DBOUND[0], DBOUND[1], DBOUND[2], dtype=np.float32)
    xs = np.linspace(0.0, iW - 1.0, fW, dtype=np.float32)
    ys = np.linspace(0.0, iH - 1.0, fH, dtype=np.float32)
    return np.stack(np.broadcast_arrays(
        xs[None, None, :], ys[None, :, None], ds[:, None, None]), -1
    ).astype(np.float32)  # [D, fH, fW, 3]


def _segments(camera_intrinsics, camera2lidar, img_aug_matrix, lidar_aug_matrix):
    """Replicates reference get_geometry + voxelization in numpy float32.
    Returns (seg[Np] int64, kept[Np] bool)."""
    intr = np.asarray(camera_intrinsics, np.float32)
    c2l = np.asarray(camera2lidar, np.float32)
    img_aug = np.asarray(img_aug_matrix, np.float32)
    lidar_aug = np.asarray(lidar_aug_matrix, np.float32)

    intrins = intr[..., :3, :3]
    post_rots = img_aug[..., :3, :3]
    post_trans = img_aug[..., :3, 3]
    rots = c2l[..., :3, :3]
    trans = c2l[..., :3, 3]
    er = lidar_aug[..., :3, :3]
    et = lidar_aug[..., :3, 3]

    f = _frustum()
    pts = f[None, None] - post_trans[:, :, None, None, None, :]
    ipr = np.linalg.inv(post_rots.astype(np.float64)).astype(np.float32)
    pts = np.einsum('bnij,bndhwj->bndhwi', ipr, pts).astype(np.float32)
    pts = np.concatenate([pts[..., :2] * pts[..., 2:3], pts[..., 2:3]], -1)
    iintr = np.linalg.inv(intrins.astype(np.float64)).astype(np.float32)
    comb = np.einsum('bnij,bnjk->bnik', rots, iintr).astype(np.float32)
    pts = (np.einsum('bnij,bndhwj->bndhwi', comb, pts)
           + trans[:, :, None, None, None, :]).astype(np.float32)
    pts = (np.einsum('bij,bndhwj->bndhwi', er, pts)
           + et[:, None, None, None, None, :]).astype(np.float32)

    Np_ = pts.size // 3
    geom = ((pts - (BX - DX / 2.0)) / DX).astype(np.int32).reshape(Np_, 3)
    kept = ((geom[:, 0] >= 0) & (geom[:, 0] < NX[0])
            & (geom[:, 1] >= 0) & (geom[:, 1] < NX[1])
            & (geom[:, 2] >= 0) & (geom[:, 2] < NX[2]))
    seg = (geom[:, 2].astype(np.int64) * (NX[0] * NX[1])
           + geom[:, 0].astype(np.int64) * NX[1]
           + geom[:, 1].astype(np.int64))
    return seg, kept


def _plan(seg, kept):
    """Sort kept points, shard across cores by original-point count, pack
    into chunks whose pseudo-point granularity depends on the window type
    (A: same-segment pairs, D: singles).

    Returns per-core rows [nchunk,P,2] (-1 pad; D chunks use [...,0] only),
    rel [nchunk,P], slot_seg [nchunk,WIN], plus global counts.
    """
    kidx = np.nonzero(kept)[0].astype(np.int64)
    segk = seg[kidx]
    order = np.argsort(segk, kind='stable')
    rows_sorted = kidx[order]
    seg_sorted = segk[order]
    counts = np.bincount(seg_sorted, minlength=NSEG).astype(np.float32)

    rs = np.flatnonzero(np.r_[True, np.diff(seg_sorted) != 0]).astype(np.int64)
    rlen = np.diff(np.r_[rs, len(seg_sorted)]).astype(np.int64)
    run_seg = seg_sorted[rs]
    npts = len(seg_sorted)
    bounds = [int(round(npts * k / NCORES)) for k in range(NCORES + 1)]

    # first pass per core: build pieces (run, take_pts, poff, slot, chunk,
    # fill0) walking original points with window-type-dependent pairing
    cores_pieces = []
    ri = 0
    pcum = np.concatenate([[0], np.cumsum(rlen)])
    for k in range(NCORES):
        lo, hi = bounds[k], bounds[k + 1]
        while ri + 1 < len(pcum) and pcum[ri + 1] <= lo:
            ri += 1
        pieces = []
        chunk = 0
        fill = 0   # pseudo slots used in current chunk
        d = 0      # distinct segs in current chunk
        p = lo
        rj = ri
        while p < hi:
            run_end = pcum[rj + 1]
            rem = int(min(run_end, hi) - p)
            poff = int(p - pcum[rj])
            while rem:
                g = 2 if _wtype(chunk // CPW) == 'A' else 1
                if fill == P:
                    chunk += 1
                    fill = 0
                    d = 0
                    continue
                if d == WIN:  # chunk out of slots: pad to chunk end
                    chunk += 1
                    fill = 0
                    d = 0
                    continue
                slot = d
                d += 1
                take = min(rem, (P - fill) * g)
                npse = (take + g - 1) // g
                pieces.append((rj, take, poff, slot, chunk, fill, g))
                fill += npse
                rem -= take
                poff += take
            p = int(min(run_end, hi))
            if p >= run_end:
                rj += 1
        cores_pieces.append((pieces, chunk + (1 if fill else 0)))

    nchunk = max(nc_ for _, nc_ in cores_pieces)
    nchunk = max(CPW, ((nchunk + CPW - 1) // CPW) * CPW)

    out = []
    for k, (pieces, _) in enumerate(cores_pieces):
        rows = np.full((nchunk, P, 2), -1, np.int64)
        rel = np.full((nchunk, P), -1, np.int32)
        slot_seg = np.full((nchunk, WIN), -1, np.int64)
        for (rj, take, poff, slot, c, fill0, g) in pieces:
            npse = (take + g - 1) // g
            vals = rows_sorted[rs[rj] + poff: rs[rj] + poff + take]
            arr = np.full(npse * g, -1, np.int64)
            arr[:take] = vals
            rows[c, fill0:fill0 + npse, :g] = arr.reshape(npse, g)
            rel[c, fill0:fill0 + npse] = slot
            slot_seg[c, slot] = run_seg[rj]
        out.append(dict(rows=rows, rel=rel, slot_seg=slot_seg))
    return dict(nchunk=nchunk, cores=out, counts=counts)


# ---------------- device program ----------------
_COMPILED = {}


def _build_program(nchunk):
    import concourse.tile as tile
    from concourse import bacc, mybir
    import concourse.bass as bass

    key = (nchunk, WIN, APAT)
    if key in _COMPILED:
        return _COMPILED[key]

    nwin = nchunk // CPW
    nwinA = len([w for w in range(nwin) if _wtype(w) == 'A'])
    nwinD = nwin - nwinA
    bf = mybir.dt.bfloat16
    i8 = mybir.dt.int8
    nc = bacc.Bacc("TRN2", target_bir_lowering=False, debug=False,
                   enable_asserts=False, num_devices=NCORES)
    f8 = mybir.dt.float8e3
    ptsa = nc.dram_tensor("ptsa", [P, max(1, nwinA * 2 * WB)], i8,
                          kind="ExternalInput").ap()
    ptsd = nc.dram_tensor("ptsd", [P, max(1, nwinD * WB)], f8,
                          kind="ExternalInput").ap()
    rel = nc.dram_tensor("rel", [P, nchunk], bf, kind="ExternalInput").ap()
    iota = nc.dram_tensor("iota", [P, WIN], bf, kind="ExternalInput").ap()
    wout = nc.dram_tensor("wout", [nwin, C_OUT, CPW * WIN], bf,
                          kind="ExternalOutput").ap()

    with tile.TileContext(nc) as tc:
        with tc.tile_pool(name="const", bufs=1) as constp, \
             tc.tile_pool(name="feata", bufs=5) as featap, \
             tc.tile_pool(name="featd", bufs=5) as featdp, \
             tc.tile_pool(name="pair", bufs=5) as pairp, \
             tc.tile_pool(name="oh", bufs=8) as ohp, \
             tc.tile_pool(name="stage", bufs=6) as stagep, \
             tc.tile_pool(name="psum", bufs=6, space="PSUM") as psump:
            rel_t = constp.tile([P, nchunk], bf)
            nc.scalar.dma_start(out=rel_t[:], in_=rel[:])
            iota_t = constp.tile([P, WIN], bf)
            nc.scalar.dma_start(out=iota_t[:], in_=iota[:])

            a_i = 0
            d_i = 0
            for w0 in range(0, nwin, APAT):
                ws = range(w0, min(w0 + APAT, nwin))
                # input DMAs for the whole period first, split across queues
                fsrc = {}
                dfirst = True
                for w in ws:
                    if _wtype(w) == 'A':
                        f_t = featap.tile([P, 2 * WB], i8)
                        nc.sync.dma_start(
                            out=f_t[:],
                            in_=ptsa[:, a_i * 2 * WB:(a_i + 1) * 2 * WB])
                        a_i += 1
                        fsrc[w] = (f_t, 0)
                    else:
                        dt = featdp.tile([P, WB + PAD], f8)
                        eng = nc.scalar if dfirst else nc.sync
                        dfirst = False
                        eng.dma_start(
                            out=dt[:, 0:WB],
                            in_=ptsd[:, d_i * WB:(d_i + 1) * WB])
                        d_i += 1
                        fsrc[w] = (dt, 0)
                # one-hots for the period (DVE; only need rel_t)
                ohs = {}
                for w in ws:
                    oh = ohp.tile([P, CPW, WIN],
                                  bf if _wtype(w) == 'A' else f8)
                    rsl = rel_t[:, w * CPW:(w + 1) * CPW]
                    rel_b = bass.AP(rsl.tensor, rsl.offset,
                                    list(rsl.ap) + [[0, WIN]])
                    iap = iota_t[:]
                    iota_b = bass.AP(iap.tensor, iap.offset,
                                     [iap.ap[0], [0, CPW], iap.ap[1]])
                    nc.vector.tensor_tensor(out=oh[:], in0=iota_b, in1=rel_b,
                                            op=mybir.AluOpType.is_equal)
                    ohs[w] = oh
                # compute per window
                for w in ws:
                    tsrc, off = fsrc[w]
                    if _wtype(w) == 'A':
                        src = pairp.tile([P, WB + PAD], bf)
                        nc.vector.tensor_tensor(
                            out=src[:, 0:WB], in0=tsrc[:, 0:WB],
                            in1=tsrc[:, WB:2 * WB], op=mybir.AluOpType.add)
                        off = 0
                    else:
                        src = tsrc
                    oh = ohs[w]
                    ps = psump.tile([P, 512], mybir.dt.float32)
                    for c in range(CPW):
                        # stationary = features padded to 128 cols (junk-read)
                        nc.tensor.matmul(
                            out=ps[0:P, c * WIN:(c + 1) * WIN],
                            lhsT=src[:, off + c * C_OUT:off + c * C_OUT + P],
                            rhs=oh[:, c],
                            start=True,
                            stop=True,
                        )
                    st = stagep.tile([C_OUT, CPW * WIN], bf)
                    nc.scalar.copy(out=st[:], in_=ps[0:C_OUT, 0:CPW * WIN])
                    nc.scalar.dma_start(out=wout[w], in_=st[:])

    nc.compile()
    _COMPILED[key] = nc
    return nc


def _run_on_hw(nc, in_maps, trace=False):
    from concourse.bass_utils import run_bass_kernel_spmd
    from concourse.bass_interp import get_hw_module

    if trace:
        try:
            import ntff_hook
            ntff_hook.install()
        except Exception:
            pass
    hw_m = get_hw_module(nc.m)
    old_m = nc.m
    nc.m = hw_m
    try:
        res = run_bass_kernel_spmd(
            nc, in_maps, core_ids=list(range(NCORES)), trace=trace,
        )
    finally:
        nc.m = old_m
    return res


def kernel(cam_feats, camera_intrinsics, camera2lidar, img_aug_matrix,
           lidar_aug_matrix, _trace=False, _return_results=False):
    cam = np.ascontiguousarray(np.asarray(cam_feats, np.float32))
    Npts = cam.size // C_OUT
    cam_f = cam.reshape(Npts, C_OUT)
    cam_aug = np.vstack([cam_f, np.zeros((1, C_OUT), np.float32)])

    seg, kept = _segments(camera_intrinsics, camera2lidar,
                          img_aug_matrix, lidar_aug_matrix)
    plan = _plan(seg, kept)
    nchunk = plan['nchunk']
    nwin = nchunk // CPW
    wtypes = np.array([_wtype(w) == 'A' for w in range(nwin)])
    a_chunks = np.repeat(wtypes, CPW)

    iota_c = np.broadcast_to(np.arange(WIN, dtype=np.float32),
                             (P, WIN)).astype(ml_dtypes.bfloat16)
    in_maps = []
    scales = []
    for k in range(NCORES):
        ck = plan['cores'][k]
        s = np.ones(nchunk, np.float32)
        # A windows: int8 pairs
        rows_a = ck['rows'][a_chunks]
        fa = cam_aug[rows_a.reshape(-1)].reshape(-1, P, 2, C_OUT)
        aa = np.abs(fa).reshape(len(fa), -1).max(axis=1)
        sa = np.where(aa > 0, aa / 127.0, 1.0).astype(np.float32)
        qa = np.clip(np.rint(fa / sa[:, None, None, None]),
                     -127, 127).astype(np.int8)
        s[a_chunks] = sa
        nwa = len(fa) // CPW
        pa = qa.reshape(nwa, CPW, P, 2, C_OUT).transpose(2, 0, 3, 1, 4)
        pa = np.ascontiguousarray(pa).reshape(P, nwa * 2 * CPW * C_OUT)
        # D windows: fp8 e3m4 direct with per-chunk scale
        rows_d = ck['rows'][~a_chunks][:, :, 0]
        fd = cam_aug[rows_d.reshape(-1)].reshape(-1, P, C_OUT)
        ad = np.abs(fd).reshape(len(fd), -1).max(axis=1)
        sd = np.where(ad > 0, ad / F8MAX, 1.0).astype(np.float32)
        qd = (fd / sd[:, None, None]).astype(ml_dtypes.float8_e3m4)
        s[~a_chunks] = sd
        nwd = len(fd) // CPW
        pd = qd.reshape(nwd, CPW, P, C_OUT).transpose(2, 0, 1, 3)
        pd = np.ascontiguousarray(pd).reshape(P, max(1, nwd * CPW * C_OUT))
        scales.append(s)
        relk = np.ascontiguousarray(
            ck['rel'].T.astype(np.float32)).astype(ml_dtypes.bfloat16)
        in_maps.append(dict(ptsa=pa, ptsd=pd, rel=relk, iota=iota_c))

    nc = _build_program(nchunk)
    res = _run_on_hw(nc, in_maps, trace=_trace)

    # ---------------- host assembly ----------------
    s_parts = []
    v_parts = []
    for k in range(NCORES):
        vals = np.asarray(res.results[k]['wout']).astype(np.float32)
        vals = vals.reshape(nwin, C_OUT, CPW, WIN).transpose(0, 2, 3, 1)
        vals = vals.reshape(nchunk, WIN, C_OUT) * scales[k][:, None, None]
        cseg = plan['cores'][k]['slot_seg']
        m = cseg >= 0
        s_parts.append(cseg[m])
        v_parts.append(vals[m])
    s_all = np.concatenate(s_parts)
    v_all = np.concatenate(v_parts)
    acc = np.zeros((NSEG, C_OUT), np.float32)
    if len(s_all):
        o2 = np.argsort(s_all, kind='stable')
        s2 = s_all[o2]
        v2 = v_all[o2]
        starts = np.r_[0, np.flatnonzero(np.diff(s2)) + 1]
        sums = np.add.reduceat(v2, starts, axis=0)
        useg = s2[starts]
        acc[useg] = sums / np.maximum(plan['counts'][useg], 1.0)[:, None]

    out = acc.reshape(NX[2], NX[0], NX[1], C_OUT).transpose(0, 3, 1, 2)
    out = out.reshape(1, NX[2] * C_OUT, NX[0], NX[1]).astype(np.float32)
    if _return_results:
        return out, res
    return out


# revision 42
# speedup vs baseline: 1.0566x; 1.0566x over previous
"""Trainium2 Bass kernel for nn_BaseViewTransform (BEVFusion bev_pool / segment-mean).

Pipeline (v3 — hybrid int8-pair / bf16-direct windows):
  The machine balance: HBM DMA wants few bytes (int8), but int8 needs an
  on-device convert to bf16 for the PE, and only the DVE can do that at
  ~1 elem/ns/partition (GPSIMD/multi-engine runs CONTEND and go slower).
  So windows alternate between two types:
    A (int8 pairs):  80 B/pt DMA + DVE pair-add (int8+int8 -> bf16, exact)
    D (bf16 direct): 160 B/pt DMA + zero DVE work (feats feed the PE
                     straight from the DMA tile)
  which equalizes the DMA wall and the DVE wall.

  Host (index plane only): compute per-point voxel ids exactly as the
  reference, sort kept points by segment, shard across 8 cores, pack into
  128-pseudo chunks (pseudo = same-segment pair in A windows, single point
  in D windows) with <= WIN=12 distinct segments per chunk; per-chunk int8
  scales for A chunks.
  Device, per 42-chunk window: DMA stream in; [A only] DVE pair-add;
  DVE one-hot via is_equal; 42 matmuls with the FEATURES as the stationary
  operand padded to 128 columns (FWL) and the one-hot moving (12 cols);
  PSUM [0:80] -> SBUF bf16 via ACT; DMA out.
  Host: scale chunk sums, reduce per segment, divide by counts, scatter.
"""

import numpy as np
import ml_dtypes

# ---------------- problem constants (hardcoded per task rules) ----------------
IMAGE_SIZE = (256, 704)
FEATURE_SIZE = (32, 88)
XBOUND = (-54.0, 54.0, 0.3)
YBOUND = (-54.0, 54.0, 0.3)
ZBOUND = (-10.0, 10.0, 20.0)
DBOUND = (1.0, 60.0, 0.5)
C_OUT = 80
NX = (360, 360, 1)
NSEG = NX[2] * NX[0] * NX[1]  # 129600
DX = np.array([XBOUND[2], YBOUND[2], ZBOUND[2]], np.float32)
BX = np.array([XBOUND[0] + XBOUND[2] / 2.0,
               YBOUND[0] + YBOUND[2] / 2.0,
               ZBOUND[0] + ZBOUND[2] / 2.0], np.float32)

NCORES = 8
P = 128          # pseudo-points per chunk (= matmul contraction dim)
WIN = 12         # max distinct segments per chunk (= one-hot width)
CPW = 42         # chunks per window (42*12 = 504 <= 512 fp32 PSUM bank)
PAD = P - C_OUT  # lhsT column padding to 128 cols so FWL triggers
WB = CPW * C_OUT  # feature elems per window block per partition (3360)
# window type pattern: 1 A window (int8 pairs) : 2 D windows (fp8 e3m4)
APAT = 3         # pattern period; w % APAT == 0 -> A window
F8MAX = 14.0     # fp8 e3m4 scaling target (max finite 15.5, margin)


def _wtype(w):
    # A window last in each period so the program starts on D windows
    # (no pair-add dependency -> matmuls start right after the first DMA)
    return 'A' if w % APAT == APAT - 1 else 'D'


def _frustum():
    iH, iW = IMAGE_SIZE
    fH, fW = FEATURE_SIZE
    ds = np.arange(DBOUND[0], DBOUND[1], DBOUND[2], dtype=np.float32)
    xs = np.linspace(0.0, iW - 1.0, fW, dtype=np.float32)
    ys = np.linspace(0.0, iH - 1.0, fH, dtype=np.float32)
    return np.stack(np.broadcast_arrays(
        xs[None, None, :], ys[None, :, None], ds[:, None, None]), -1
    ).astype(np.float32)  # [D, fH, fW, 3]


def _segments(camera_intrinsics, camera2lidar, img_aug_matrix, lidar_aug_matrix):
    """Replicates reference get_geometry + voxelization in numpy float32.
    Returns (seg[Np] int64, kept[Np] bool)."""
    intr = np.asarray(camera_intrinsics, np.float32)
    c2l = np.asarray(camera2lidar, np.float32)
    img_aug = np.asarray(img_aug_matrix, np.float32)
    lidar_aug = np.asarray(lidar_aug_matrix, np.float32)

    intrins = intr[..., :3, :3]
    post_rots = img_aug[..., :3, :3]
    post_trans = img_aug[..., :3, 3]
    rots = c2l[..., :3, :3]
    trans = c2l[..., :3, 3]
    er = lidar_aug[..., :3, :3]
    et = lidar_aug[..., :3, 3]

    f = _frustum()
    pts = f[None, None] - post_trans[:, :, None, None, None, :]
    ipr = np.linalg.inv(post_rots.astype(np.float64)).astype(np.float32)
    pts = np.einsum('bnij,bndhwj->bndhwi', ipr, pts).astype(np.float32)
    pts = np.concatenate([pts[..., :2] * pts[..., 2:3], pts[..., 2:3]], -1)
    iintr = np.linalg.inv(intrins.astype(np.float64)).astype(np.float32)
    comb = np.einsum('bnij,bnjk->bnik', rots, iintr).astype(np.float32)
    pts = (np.einsum('bnij,bndhwj->bndhwi', comb, pts)
           + trans[:, :, None, None, None, :]).astype(np.float32)
    pts = (np.einsum('bij,bndhwj->bndhwi', er, pts)
           + et[:, None, None, None, None, :]).astype(np.float32)

    Np_ = pts.size // 3
    geom = ((pts - (BX - DX / 2.0)) / DX).astype(np.int32).reshape(Np_, 3)
    kept = ((geom[:, 0] >= 0) & (geom[:, 0] < NX[0])
            & (geom[:, 1] >= 0) & (geom[:, 1] < NX[1])
            & (geom[:, 2] >= 0) & (geom[:, 2] < NX[2]))
    seg = (geom[:, 2].astype(np.int64) * (NX[0] * NX[1])
           + geom[:, 0].astype(np.int64) * NX[1]
           + geom[:, 1].astype(np.int64))
    return seg, kept


def _plan(seg, kept):
    """Sort kept points, shard across cores by original-point count, pack
    into chunks whose pseudo-point granularity depends on the window type
    (A: same-segment pairs, D: singles).

    Returns per-core rows [nchunk,P,2] (-1 pad; D chunks use [...,0] only),
    rel [nchunk,P], slot_seg [nchunk,WIN], plus global counts.
    """
    kidx = np.nonzero(kept)[0].astype(np.int64)
    segk = seg[kidx]
    order = np.argsort(segk, kind='stable')
    rows_sorted = kidx[order]
    seg_sorted = segk[order]
    counts = np.bincount(seg_sorted, minlength=NSEG).astype(np.float32)

    rs = np.flatnonzero(np.r_[True, np.diff(seg_sorted) != 0]).astype(np.int64)
    rlen = np.diff(np.r_[rs, len(seg_sorted)]).astype(np.int64)
    run_seg = seg_sorted[rs]
    npts = len(seg_sorted)
    bounds = [int(round(npts * k / NCORES)) for k in range(NCORES + 1)]

    # first pass per core: build pieces (run, take_pts, poff, slot, chunk,
    # fill0) walking original points with window-type-dependent pairing
    cores_pieces = []
    ri = 0
    pcum = np.concatenate([[0], np.cumsum(rlen)])
    for k in range(NCORES):
        lo, hi = bounds[k], bounds[k + 1]
        while ri + 1 < len(pcum) and pcum[ri + 1] <= lo:
            ri += 1
        pieces = []
        chunk = 0
        fill = 0   # pseudo slots used in current chunk
        d = 0      # distinct segs in current chunk
        p = lo
        rj = ri
        while p < hi:
            run_end = pcum[rj + 1]
            rem = int(min(run_end, hi) - p)
            poff = int(p - pcum[rj])
            while rem:
                g = 2 if _wtype(chunk // CPW) == 'A' else 1
                if fill == P:
                    chunk += 1
                    fill = 0
                    d = 0
                    continue
                if d == WIN:  # chunk out of slots: pad to chunk end
                    chunk += 1
                    fill = 0
                    d = 0
                    continue
                slot = d
                d += 1
                take = min(rem, (P - fill) * g)
                npse = (take + g - 1) // g
                pieces.append((rj, take, poff, slot, chunk, fill, g))
                fill += npse
                rem -= take
                poff += take
            p = int(min(run_end, hi))
            if p >= run_end:
                rj += 1
        cores_pieces.append((pieces, chunk + (1 if fill else 0)))

    nchunk = max(nc_ for _, nc_ in cores_pieces)
    nchunk = max(CPW, ((nchunk + CPW - 1) // CPW) * CPW)

    out = []
    for k, (pieces, _) in enumerate(cores_pieces):
        rows = np.full((nchunk, P, 2), -1, np.int64)
        rel = np.full((nchunk, P), -1, np.int32)
        slot_seg = np.full((nchunk, WIN), -1, np.int64)
        for (rj, take, poff, slot, c, fill0, g) in pieces:
            npse = (take + g - 1) // g
            vals = rows_sorted[rs[rj] + poff: rs[rj] + poff + take]
            arr = np.full(npse * g, -1, np.int64)
            arr[:take] = vals
            rows[c, fill0:fill0 + npse, :g] = arr.reshape(npse, g)
            rel[c, fill0:fill0 + npse] = slot
            slot_seg[c, slot] = run_seg[rj]
        out.append(dict(rows=rows, rel=rel, slot_seg=slot_seg))
    return dict(nchunk=nchunk, cores=out, counts=counts)


# ---------------- device program ----------------
_COMPILED = {}


def _build_program(nchunk):
    import concourse.tile as tile
    from concourse import bacc, mybir
    import concourse.bass as bass

    key = (nchunk, WIN, APAT)
    if key in _COMPILED:
        return _COMPILED[key]

    nwin = nchunk // CPW
    nwinA = len([w for w in range(nwin) if _wtype(w) == 'A'])
    nwinD = nwin - nwinA
    bf = mybir.dt.bfloat16
    i8 = mybir.dt.int8
    nc = bacc.Bacc("TRN2", target_bir_lowering=False, debug=False,
                   enable_asserts=False, num_devices=NCORES)
    f8 = mybir.dt.float8e3
    ptsa = nc.dram_tensor("ptsa", [P, max(1, nwinA * 2 * WB)], i8,
                          kind="ExternalInput").ap()
    ptsd = nc.dram_tensor("ptsd", [P, max(1, nwinD * WB)], f8,
                          kind="ExternalInput").ap()
    rel = nc.dram_tensor("rel", [P, nchunk], bf, kind="ExternalInput").ap()
    iota = nc.dram_tensor("iota", [P, WIN], bf, kind="ExternalInput").ap()
    wout = nc.dram_tensor("wout", [nwin, C_OUT, CPW * WIN], bf,
                          kind="ExternalOutput").ap()

    with tile.TileContext(nc) as tc:
        with tc.tile_pool(name="const", bufs=1) as constp, \
             tc.tile_pool(name="feata", bufs=5) as featap, \
             tc.tile_pool(name="featd", bufs=5) as featdp, \
             tc.tile_pool(name="pair", bufs=5) as pairp, \
             tc.tile_pool(name="oh", bufs=8) as ohp, \
             tc.tile_pool(name="stage", bufs=6) as stagep, \
             tc.tile_pool(name="psum", bufs=6, space="PSUM") as psump:
            rel_t = constp.tile([P, nchunk], bf)
            nc.scalar.dma_start(out=rel_t[:], in_=rel[:])
            iota_t = constp.tile([P, WIN], bf)
            nc.scalar.dma_start(out=iota_t[:], in_=iota[:])

            a_i = 0
            d_i = 0
            for w0 in range(0, nwin, APAT):
                ws = range(w0, min(w0 + APAT, nwin))
                # input DMAs for the whole period first, split across queues
                fsrc = {}
                dfirst = True
                for w in ws:
                    if _wtype(w) == 'A':
                        f_t = featap.tile([P, 2 * WB], i8)
                        nc.sync.dma_start(
                            out=f_t[:],
                            in_=ptsa[:, a_i * 2 * WB:(a_i + 1) * 2 * WB])
                        a_i += 1
                        fsrc[w] = (f_t, 0)
                    else:
                        dt = featdp.tile([P, WB + PAD], f8)
                        eng = nc.scalar if dfirst else nc.sync
                        dfirst = False
                        eng.dma_start(
                            out=dt[:, 0:WB],
                            in_=ptsd[:, d_i * WB:(d_i + 1) * WB])
                        d_i += 1
                        fsrc[w] = (dt, 0)
                # one-hots for the period (DVE; only need rel_t)
                ohs = {}
                for w in ws:
                    oh = ohp.tile([P, CPW, WIN],
                                  bf if _wtype(w) == 'A' else f8)
                    rsl = rel_t[:, w * CPW:(w + 1) * CPW]
                    rel_b = bass.AP(rsl.tensor, rsl.offset,
                                    list(rsl.ap) + [[0, WIN]])
                    iap = iota_t[:]
                    iota_b = bass.AP(iap.tensor, iap.offset,
                                     [iap.ap[0], [0, CPW], iap.ap[1]])
                    nc.vector.tensor_tensor(out=oh[:], in0=iota_b, in1=rel_b,
                                            op=mybir.AluOpType.is_equal)
                    ohs[w] = oh
                # compute per window
                for w in ws:
                    tsrc, off = fsrc[w]
                    if _wtype(w) == 'A':
                        src = pairp.tile([P, WB + PAD], bf)
                        nc.vector.tensor_tensor(
                            out=src[:, 0:WB], in0=tsrc[:, 0:WB],
                            in1=tsrc[:, WB:2 * WB], op=mybir.AluOpType.add)
                        off = 0
                    else:
                        src = tsrc
                    oh = ohs[w]
                    ps = psump.tile([P, 512], mybir.dt.float32)
                    for c in range(CPW):
                        # stationary = features padded to 128 cols (junk-read)
                        nc.tensor.matmul(
                            out=ps[0:P, c * WIN:(c + 1) * WIN],
                            lhsT=src[:, off + c * C_OUT:off + c * C_OUT + P],
                            rhs=oh[:, c],
                            start=True,
                            stop=True,
                        )
                    st = stagep.tile([C_OUT, CPW * WIN], bf)
                    nc.scalar.copy(out=st[:], in_=ps[0:C_OUT, 0:CPW * WIN])
                    nc.scalar.dma_start(out=wout[w], in_=st[:])

    nc.compile()
    _COMPILED[key] = nc
    return nc


def _run_on_hw(nc, in_maps, trace=False):
    from concourse.bass_utils import run_bass_kernel_spmd
    from concourse.bass_interp import get_hw_module

    if trace:
        try:
            import ntff_hook
            ntff_hook.install()
        except Exception:
            pass
    hw_m = get_hw_module(nc.m)
    old_m = nc.m
    nc.m = hw_m
    try:
        res = run_bass_kernel_spmd(
            nc, in_maps, core_ids=list(range(NCORES)), trace=trace,
        )
    finally:
        nc.m = old_m
    return res


def kernel(cam_feats, camera_intrinsics, camera2lidar, img_aug_matrix,
           lidar_aug_matrix, _trace=False, _return_results=False):
    cam = np.ascontiguousarray(np.asarray(cam_feats, np.float32))
    Npts = cam.size // C_OUT
    cam_f = cam.reshape(Npts, C_OUT)
    cam_aug = np.vstack([cam_f, np.zeros((1, C_OUT), np.float32)])

    seg, kept = _segments(camera_intrinsics, camera2lidar,
                          img_aug_matrix, lidar_aug_matrix)
    plan = _plan(seg, kept)
    nchunk = plan['nchunk']
    nwin = nchunk // CPW
    wtypes = np.array([_wtype(w) == 'A' for w in range(nwin)])
    a_chunks = np.repeat(wtypes, CPW)

    iota_c = np.broadcast_to(np.arange(WIN, dtype=np.float32),
                             (P, WIN)).astype(ml_dtypes.bfloat16)
    in_maps = []
    scales = []
    for k in range(NCORES):
        ck = plan['cores'][k]
        s = np.ones(nchunk, np.float32)
        # A windows: int8 pairs
        rows_a = ck['rows'][a_chunks]
        fa = cam_aug[rows_a.reshape(-1)].reshape(-1, P, 2, C_OUT)
        aa = np.abs(fa).reshape(len(fa), -1).max(axis=1)
        sa = np.where(aa > 0, aa / 127.0, 1.0).astype(np.float32)
        qa = np.clip(np.rint(fa / sa[:, None, None, None]),
                     -127, 127).astype(np.int8)
        s[a_chunks] = sa
        nwa = len(fa) // CPW
        pa = qa.reshape(nwa, CPW, P, 2, C_OUT).transpose(2, 0, 3, 1, 4)
        pa = np.ascontiguousarray(pa).reshape(P, nwa * 2 * CPW * C_OUT)
        # D windows: fp8 e3m4 direct with per-chunk scale
        rows_d = ck['rows'][~a_chunks][:, :, 0]
        fd = cam_aug[rows_d.reshape(-1)].reshape(-1, P, C_OUT)
        ad = np.abs(fd).reshape(len(fd), -1).max(axis=1)
        sd = np.where(ad > 0, ad / F8MAX, 1.0).astype(np.float32)
        qd = (fd / sd[:, None, None]).astype(ml_dtypes.float8_e3m4)
        s[~a_chunks] = sd
        nwd = len(fd) // CPW
        pd = qd.reshape(nwd, CPW, P, C_OUT).transpose(2, 0, 1, 3)
        pd = np.ascontiguousarray(pd).reshape(P, max(1, nwd * CPW * C_OUT))
        scales.append(s)
        relk = np.ascontiguousarray(
            ck['rel'].T.astype(np.float32)).astype(ml_dtypes.bfloat16)
        in_maps.append(dict(ptsa=pa, ptsd=pd, rel=relk, iota=iota_c))

    nc = _build_program(nchunk)
    res = _run_on_hw(nc, in_maps, trace=_trace)

    # ---------------- host assembly ----------------
    s_parts = []
    v_parts = []
    for k in range(NCORES):
        vals = np.asarray(res.results[k]['wout']).astype(np.float32)
        vals = vals.reshape(nwin, C_OUT, CPW, WIN).transpose(0, 2, 3, 1)
        vals = vals.reshape(nchunk, WIN, C_OUT) * scales[k][:, None, None]
        cseg = plan['cores'][k]['slot_seg']
        m = cseg >= 0
        s_parts.append(cseg[m])
        v_parts.append(vals[m])
    s_all = np.concatenate(s_parts)
    v_all = np.concatenate(v_parts)
    acc = np.zeros((NSEG, C_OUT), np.float32)
    if len(s_all):
        o2 = np.argsort(s_all, kind='stable')
        s2 = s_all[o2]
        v2 = v_all[o2]
        starts = np.r_[0, np.flatnonzero(np.diff(s2)) + 1]
        sums = np.add.reduceat(v2, starts, axis=0)
        useg = s2[starts]
        acc[useg] = sums / np.maximum(plan['counts'][useg], 1.0)[:, None]

    out = acc.reshape(NX[2], NX[0], NX[1], C_OUT).transpose(0, 3, 1, 2)
    out = out.reshape(1, NX[2] * C_OUT, NX[0], NX[1]).astype(np.float32)
    if _return_results:
        return out, res
    return out


# revision 43
# speedup vs baseline: 1.1650x; 1.1026x over previous
"""Trainium2 Bass kernel for nn_BaseViewTransform (BEVFusion bev_pool / segment-mean).

Pipeline (v3 — hybrid int8-pair / bf16-direct windows):
  The machine balance: HBM DMA wants few bytes (int8), but int8 needs an
  on-device convert to bf16 for the PE, and only the DVE can do that at
  ~1 elem/ns/partition (GPSIMD/multi-engine runs CONTEND and go slower).
  So windows alternate between two types:
    A (int8 pairs):  80 B/pt DMA + DVE pair-add (int8+int8 -> bf16, exact)
    D (bf16 direct): 160 B/pt DMA + zero DVE work (feats feed the PE
                     straight from the DMA tile)
  which equalizes the DMA wall and the DVE wall.

  Host (index plane only): compute per-point voxel ids exactly as the
  reference, sort kept points by segment, shard across 8 cores, pack into
  128-pseudo chunks (pseudo = same-segment pair in A windows, single point
  in D windows) with <= WIN=12 distinct segments per chunk; per-chunk int8
  scales for A chunks.
  Device, per 42-chunk window: DMA stream in; [A only] DVE pair-add;
  DVE one-hot via is_equal; 42 matmuls with the FEATURES as the stationary
  operand padded to 128 columns (FWL) and the one-hot moving (12 cols);
  PSUM [0:80] -> SBUF bf16 via ACT; DMA out.
  Host: scale chunk sums, reduce per segment, divide by counts, scatter.
"""

import numpy as np
import ml_dtypes

# ---------------- problem constants (hardcoded per task rules) ----------------
IMAGE_SIZE = (256, 704)
FEATURE_SIZE = (32, 88)
XBOUND = (-54.0, 54.0, 0.3)
YBOUND = (-54.0, 54.0, 0.3)
ZBOUND = (-10.0, 10.0, 20.0)
DBOUND = (1.0, 60.0, 0.5)
C_OUT = 80
NX = (360, 360, 1)
NSEG = NX[2] * NX[0] * NX[1]  # 129600
DX = np.array([XBOUND[2], YBOUND[2], ZBOUND[2]], np.float32)
BX = np.array([XBOUND[0] + XBOUND[2] / 2.0,
               YBOUND[0] + YBOUND[2] / 2.0,
               ZBOUND[0] + ZBOUND[2] / 2.0], np.float32)

NCORES = 8
P = 128          # pseudo-points per chunk (= matmul contraction dim)
WIN = 12         # max distinct segments per chunk (= one-hot width)
CPW = 42         # chunks per window (42*12 = 504 <= 512 fp32 PSUM bank)
PAD = P - C_OUT  # lhsT column padding to 128 cols so FWL triggers
WB = CPW * C_OUT  # feature elems per window block per partition (3360)
# window type pattern: 1 A window (int8 pairs) : 2 D windows (fp8 e3m4)
APAT = 3         # pattern period; w % APAT == 0 -> A window
F8MAX = 14.0     # fp8 e3m4 scaling target (max finite 15.5, margin)


def _wtype(w):
    # A window last in each period so the program starts on D windows
    # (no pair-add dependency -> matmuls start right after the first DMA)
    return 'A' if w % APAT == APAT - 1 else 'D'


def _frustum():
    iH, iW = IMAGE_SIZE
    fH, fW = FEATURE_SIZE
    ds = np.arange(DBOUND[0], DBOUND[1], DBOUND[2], dtype=np.float32)
    xs = np.linspace(0.0, iW - 1.0, fW, dtype=np.float32)
    ys = np.linspace(0.0, iH - 1.0, fH, dtype=np.float32)
    return np.stack(np.broadcast_arrays(
        xs[None, None, :], ys[None, :, None], ds[:, None, None]), -1
    ).astype(np.float32)  # [D, fH, fW, 3]


def _segments(camera_intrinsics, camera2lidar, img_aug_matrix, lidar_aug_matrix):
    """Replicates reference get_geometry + voxelization in numpy float32.
    Returns (seg[Np] int64, kept[Np] bool)."""
    intr = np.asarray(camera_intrinsics, np.float32)
    c2l = np.asarray(camera2lidar, np.float32)
    img_aug = np.asarray(img_aug_matrix, np.float32)
    lidar_aug = np.asarray(lidar_aug_matrix, np.float32)

    intrins = intr[..., :3, :3]
    post_rots = img_aug[..., :3, :3]
    post_trans = img_aug[..., :3, 3]
    rots = c2l[..., :3, :3]
    trans = c2l[..., :3, 3]
    er = lidar_aug[..., :3, :3]
    et = lidar_aug[..., :3, 3]

    f = _frustum()
    pts = f[None, None] - post_trans[:, :, None, None, None, :]
    ipr = np.linalg.inv(post_rots.astype(np.float64)).astype(np.float32)
    pts = np.einsum('bnij,bndhwj->bndhwi', ipr, pts).astype(np.float32)
    pts = np.concatenate([pts[..., :2] * pts[..., 2:3], pts[..., 2:3]], -1)
    iintr = np.linalg.inv(intrins.astype(np.float64)).astype(np.float32)
    comb = np.einsum('bnij,bnjk->bnik', rots, iintr).astype(np.float32)
    pts = (np.einsum('bnij,bndhwj->bndhwi', comb, pts)
           + trans[:, :, None, None, None, :]).astype(np.float32)
    pts = (np.einsum('bij,bndhwj->bndhwi', er, pts)
           + et[:, None, None, None, None, :]).astype(np.float32)

    Np_ = pts.size // 3
    geom = ((pts - (BX - DX / 2.0)) / DX).astype(np.int32).reshape(Np_, 3)
    kept = ((geom[:, 0] >= 0) & (geom[:, 0] < NX[0])
            & (geom[:, 1] >= 0) & (geom[:, 1] < NX[1])
            & (geom[:, 2] >= 0) & (geom[:, 2] < NX[2]))
    seg = (geom[:, 2].astype(np.int64) * (NX[0] * NX[1])
           + geom[:, 0].astype(np.int64) * NX[1]
           + geom[:, 1].astype(np.int64))
    return seg, kept


def _plan(seg, kept):
    """Sort kept points, shard across cores by original-point count, pack
    into chunks whose pseudo-point granularity depends on the window type
    (A: same-segment pairs, D: singles).

    Returns per-core rows [nchunk,P,2] (-1 pad; D chunks use [...,0] only),
    rel [nchunk,P], slot_seg [nchunk,WIN], plus global counts.
    """
    kidx = np.nonzero(kept)[0].astype(np.int64)
    segk = seg[kidx]
    order = np.argsort(segk, kind='stable')
    rows_sorted = kidx[order]
    seg_sorted = segk[order]
    counts = np.bincount(seg_sorted, minlength=NSEG).astype(np.float32)

    rs = np.flatnonzero(np.r_[True, np.diff(seg_sorted) != 0]).astype(np.int64)
    rlen = np.diff(np.r_[rs, len(seg_sorted)]).astype(np.int64)
    run_seg = seg_sorted[rs]
    npts = len(seg_sorted)
    bounds = [int(round(npts * k / NCORES)) for k in range(NCORES + 1)]

    # first pass per core: build pieces (run, take_pts, poff, slot, chunk,
    # fill0) walking original points with window-type-dependent pairing
    cores_pieces = []
    ri = 0
    pcum = np.concatenate([[0], np.cumsum(rlen)])
    for k in range(NCORES):
        lo, hi = bounds[k], bounds[k + 1]
        while ri + 1 < len(pcum) and pcum[ri + 1] <= lo:
            ri += 1
        pieces = []
        chunk = 0
        fill = 0   # pseudo slots used in current chunk
        d = 0      # distinct segs in current chunk
        p = lo
        rj = ri
        while p < hi:
            run_end = pcum[rj + 1]
            rem = int(min(run_end, hi) - p)
            poff = int(p - pcum[rj])
            while rem:
                g = 2 if _wtype(chunk // CPW) == 'A' else 1
                if fill == P:
                    chunk += 1
                    fill = 0
                    d = 0
                    continue
                if d == WIN:  # chunk out of slots: pad to chunk end
                    chunk += 1
                    fill = 0
                    d = 0
                    continue
                slot = d
                d += 1
                take = min(rem, (P - fill) * g)
                npse = (take + g - 1) // g
                pieces.append((rj, take, poff, slot, chunk, fill, g))
                fill += npse
                rem -= take
                poff += take
            p = int(min(run_end, hi))
            if p >= run_end:
                rj += 1
        cores_pieces.append((pieces, chunk + (1 if fill else 0)))

    nchunk = max(nc_ for _, nc_ in cores_pieces)
    nchunk = max(CPW, ((nchunk + CPW - 1) // CPW) * CPW)

    out = []
    for k, (pieces, _) in enumerate(cores_pieces):
        rows = np.full((nchunk, P, 2), -1, np.int64)
        rel = np.full((nchunk, P), -1, np.int32)
        slot_seg = np.full((nchunk, WIN), -1, np.int64)
        for (rj, take, poff, slot, c, fill0, g) in pieces:
            npse = (take + g - 1) // g
            vals = rows_sorted[rs[rj] + poff: rs[rj] + poff + take]
            arr = np.full(npse * g, -1, np.int64)
            arr[:take] = vals
            rows[c, fill0:fill0 + npse, :g] = arr.reshape(npse, g)
            rel[c, fill0:fill0 + npse] = slot
            slot_seg[c, slot] = run_seg[rj]
        out.append(dict(rows=rows, rel=rel, slot_seg=slot_seg))
    return dict(nchunk=nchunk, cores=out, counts=counts)


# ---------------- device program ----------------
_COMPILED = {}


def _build_program(nchunk):
    import concourse.tile as tile
    from concourse import bacc, mybir
    import concourse.bass as bass

    key = (nchunk, WIN, APAT)
    if key in _COMPILED:
        return _COMPILED[key]

    nwin = nchunk // CPW
    nwinA = len([w for w in range(nwin) if _wtype(w) == 'A'])
    nwinD = nwin - nwinA
    bf = mybir.dt.bfloat16
    i8 = mybir.dt.int8
    nc = bacc.Bacc("TRN2", target_bir_lowering=False, debug=False,
                   enable_asserts=False, num_devices=NCORES)
    f8 = mybir.dt.float8e3
    ptsa = nc.dram_tensor("ptsa", [P, max(1, nwinA * 2 * WB)], i8,
                          kind="ExternalInput").ap()
    ptsd = nc.dram_tensor("ptsd", [P, max(1, nwinD * WB)], f8,
                          kind="ExternalInput").ap()
    rel = nc.dram_tensor("rel", [P, nchunk], bf, kind="ExternalInput").ap()
    iota = nc.dram_tensor("iota", [P, WIN], bf, kind="ExternalInput").ap()
    wout = nc.dram_tensor("wout", [nwin, C_OUT, CPW * WIN], bf,
                          kind="ExternalOutput").ap()

    with tile.TileContext(nc) as tc:
        with tc.tile_pool(name="const", bufs=1) as constp, \
             tc.tile_pool(name="feata", bufs=6) as featap, \
             tc.tile_pool(name="featd", bufs=6) as featdp, \
             tc.tile_pool(name="pair", bufs=6) as pairp, \
             tc.tile_pool(name="oh", bufs=8) as ohp, \
             tc.tile_pool(name="stage", bufs=8) as stagep, \
             tc.tile_pool(name="psum", bufs=8, space="PSUM") as psump:
            rel_t = constp.tile([P, nchunk], bf)
            nc.scalar.dma_start(out=rel_t[:], in_=rel[:])
            iota_t = constp.tile([P, WIN], bf)
            nc.scalar.dma_start(out=iota_t[:], in_=iota[:])

            a_i = 0
            d_i = 0
            for w0 in range(0, nwin, APAT):
                ws = range(w0, min(w0 + APAT, nwin))
                # input DMAs for the whole period first, split across queues
                fsrc = {}
                dfirst = True
                for w in ws:
                    if _wtype(w) == 'A':
                        f_t = featap.tile([P, 2 * WB], i8)
                        nc.sync.dma_start(
                            out=f_t[:],
                            in_=ptsa[:, a_i * 2 * WB:(a_i + 1) * 2 * WB])
                        a_i += 1
                        fsrc[w] = (f_t, 0)
                    else:
                        dt = featdp.tile([P, WB + PAD], f8)
                        eng = nc.scalar if dfirst else nc.sync
                        dfirst = False
                        eng.dma_start(
                            out=dt[:, 0:WB],
                            in_=ptsd[:, d_i * WB:(d_i + 1) * WB])
                        d_i += 1
                        fsrc[w] = (dt, 0)
                # one-hots for the period (DVE; only need rel_t)
                ohs = {}
                for w in ws:
                    oh = ohp.tile([P, CPW, WIN],
                                  bf if _wtype(w) == 'A' else f8)
                    rsl = rel_t[:, w * CPW:(w + 1) * CPW]
                    rel_b = bass.AP(rsl.tensor, rsl.offset,
                                    list(rsl.ap) + [[0, WIN]])
                    iap = iota_t[:]
                    iota_b = bass.AP(iap.tensor, iap.offset,
                                     [iap.ap[0], [0, CPW], iap.ap[1]])
                    nc.vector.tensor_tensor(out=oh[:], in0=iota_b, in1=rel_b,
                                            op=mybir.AluOpType.is_equal)
                    ohs[w] = oh
                # compute per window
                for w in ws:
                    tsrc, off = fsrc[w]
                    if _wtype(w) == 'A':
                        src = pairp.tile([P, WB + PAD], bf)
                        nc.vector.tensor_tensor(
                            out=src[:, 0:WB], in0=tsrc[:, 0:WB],
                            in1=tsrc[:, WB:2 * WB], op=mybir.AluOpType.add)
                        off = 0
                    else:
                        src = tsrc
                    oh = ohs[w]
                    ps = psump.tile([P, 512], mybir.dt.float32)
                    for c in range(CPW):
                        # stationary = features padded to 128 cols (junk-read)
                        nc.tensor.matmul(
                            out=ps[0:P, c * WIN:(c + 1) * WIN],
                            lhsT=src[:, off + c * C_OUT:off + c * C_OUT + P],
                            rhs=oh[:, c],
                            start=True,
                            stop=True,
                        )
                    st = stagep.tile([C_OUT, CPW * WIN], bf)
                    nc.scalar.copy(out=st[:], in_=ps[0:C_OUT, 0:CPW * WIN])
                    nc.scalar.dma_start(out=wout[w], in_=st[:])

    nc.compile()
    _COMPILED[key] = nc
    return nc


def _run_on_hw(nc, in_maps, trace=False):
    from concourse.bass_utils import run_bass_kernel_spmd
    from concourse.bass_interp import get_hw_module

    if trace:
        try:
            import ntff_hook
            ntff_hook.install()
        except Exception:
            pass
    hw_m = get_hw_module(nc.m)
    old_m = nc.m
    nc.m = hw_m
    try:
        res = run_bass_kernel_spmd(
            nc, in_maps, core_ids=list(range(NCORES)), trace=trace,
        )
    finally:
        nc.m = old_m
    return res


def kernel(cam_feats, camera_intrinsics, camera2lidar, img_aug_matrix,
           lidar_aug_matrix, _trace=False, _return_results=False):
    cam = np.ascontiguousarray(np.asarray(cam_feats, np.float32))
    Npts = cam.size // C_OUT
    cam_f = cam.reshape(Npts, C_OUT)
    cam_aug = np.vstack([cam_f, np.zeros((1, C_OUT), np.float32)])

    seg, kept = _segments(camera_intrinsics, camera2lidar,
                          img_aug_matrix, lidar_aug_matrix)
    plan = _plan(seg, kept)
    nchunk = plan['nchunk']
    nwin = nchunk // CPW
    wtypes = np.array([_wtype(w) == 'A' for w in range(nwin)])
    a_chunks = np.repeat(wtypes, CPW)

    iota_c = np.broadcast_to(np.arange(WIN, dtype=np.float32),
                             (P, WIN)).astype(ml_dtypes.bfloat16)
    in_maps = []
    scales = []
    for k in range(NCORES):
        ck = plan['cores'][k]
        s = np.ones(nchunk, np.float32)
        # A windows: int8 pairs
        rows_a = ck['rows'][a_chunks]
        fa = cam_aug[rows_a.reshape(-1)].reshape(-1, P, 2, C_OUT)
        aa = np.abs(fa).reshape(len(fa), -1).max(axis=1)
        sa = np.where(aa > 0, aa / 127.0, 1.0).astype(np.float32)
        qa = np.clip(np.rint(fa / sa[:, None, None, None]),
                     -127, 127).astype(np.int8)
        s[a_chunks] = sa
        nwa = len(fa) // CPW
        pa = qa.reshape(nwa, CPW, P, 2, C_OUT).transpose(2, 0, 3, 1, 4)
        pa = np.ascontiguousarray(pa).reshape(P, nwa * 2 * CPW * C_OUT)
        # D windows: fp8 e3m4 direct with per-chunk scale
        rows_d = ck['rows'][~a_chunks][:, :, 0]
        fd = cam_aug[rows_d.reshape(-1)].reshape(-1, P, C_OUT)
        ad = np.abs(fd).reshape(len(fd), -1).max(axis=1)
        sd = np.where(ad > 0, ad / F8MAX, 1.0).astype(np.float32)
        qd = (fd / sd[:, None, None]).astype(ml_dtypes.float8_e3m4)
        s[~a_chunks] = sd
        nwd = len(fd) // CPW
        pd = qd.reshape(nwd, CPW, P, C_OUT).transpose(2, 0, 1, 3)
        pd = np.ascontiguousarray(pd).reshape(P, max(1, nwd * CPW * C_OUT))
        scales.append(s)
        relk = np.ascontiguousarray(
            ck['rel'].T.astype(np.float32)).astype(ml_dtypes.bfloat16)
        in_maps.append(dict(ptsa=pa, ptsd=pd, rel=relk, iota=iota_c))

    nc = _build_program(nchunk)
    res = _run_on_hw(nc, in_maps, trace=_trace)

    # ---------------- host assembly ----------------
    s_parts = []
    v_parts = []
    for k in range(NCORES):
        vals = np.asarray(res.results[k]['wout']).astype(np.float32)
        vals = vals.reshape(nwin, C_OUT, CPW, WIN).transpose(0, 2, 3, 1)
        vals = vals.reshape(nchunk, WIN, C_OUT) * scales[k][:, None, None]
        cseg = plan['cores'][k]['slot_seg']
        m = cseg >= 0
        s_parts.append(cseg[m])
        v_parts.append(vals[m])
    s_all = np.concatenate(s_parts)
    v_all = np.concatenate(v_parts)
    acc = np.zeros((NSEG, C_OUT), np.float32)
    if len(s_all):
        o2 = np.argsort(s_all, kind='stable')
        s2 = s_all[o2]
        v2 = v_all[o2]
        starts = np.r_[0, np.flatnonzero(np.diff(s2)) + 1]
        sums = np.add.reduceat(v2, starts, axis=0)
        useg = s2[starts]
        acc[useg] = sums / np.maximum(plan['counts'][useg], 1.0)[:, None]

    out = acc.reshape(NX[2], NX[0], NX[1], C_OUT).transpose(0, 3, 1, 2)
    out = out.reshape(1, NX[2] * C_OUT, NX[0], NX[1]).astype(np.float32)
    if _return_results:
        return out, res
    return out


# revision 44
# speedup vs baseline: 1.1843x; 1.0166x over previous
"""Trainium2 Bass kernel for nn_BaseViewTransform (BEVFusion bev_pool / segment-mean).

Pipeline (v3 — hybrid int8-pair / bf16-direct windows):
  The machine balance: HBM DMA wants few bytes (int8), but int8 needs an
  on-device convert to bf16 for the PE, and only the DVE can do that at
  ~1 elem/ns/partition (GPSIMD/multi-engine runs CONTEND and go slower).
  So windows alternate between two types:
    A (int8 pairs):  80 B/pt DMA + DVE pair-add (int8+int8 -> bf16, exact)
    D (bf16 direct): 160 B/pt DMA + zero DVE work (feats feed the PE
                     straight from the DMA tile)
  which equalizes the DMA wall and the DVE wall.

  Host (index plane only): compute per-point voxel ids exactly as the
  reference, sort kept points by segment, shard across 8 cores, pack into
  128-pseudo chunks (pseudo = same-segment pair in A windows, single point
  in D windows) with <= WIN=12 distinct segments per chunk; per-chunk int8
  scales for A chunks.
  Device, per 42-chunk window: DMA stream in; [A only] DVE pair-add;
  DVE one-hot via is_equal; 42 matmuls with the FEATURES as the stationary
  operand padded to 128 columns (FWL) and the one-hot moving (12 cols);
  PSUM [0:80] -> SBUF bf16 via ACT; DMA out.
  Host: scale chunk sums, reduce per segment, divide by counts, scatter.
"""

import numpy as np
import ml_dtypes

# ---------------- problem constants (hardcoded per task rules) ----------------
IMAGE_SIZE = (256, 704)
FEATURE_SIZE = (32, 88)
XBOUND = (-54.0, 54.0, 0.3)
YBOUND = (-54.0, 54.0, 0.3)
ZBOUND = (-10.0, 10.0, 20.0)
DBOUND = (1.0, 60.0, 0.5)
C_OUT = 80
NX = (360, 360, 1)
NSEG = NX[2] * NX[0] * NX[1]  # 129600
DX = np.array([XBOUND[2], YBOUND[2], ZBOUND[2]], np.float32)
BX = np.array([XBOUND[0] + XBOUND[2] / 2.0,
               YBOUND[0] + YBOUND[2] / 2.0,
               ZBOUND[0] + ZBOUND[2] / 2.0], np.float32)

NCORES = 8
P = 128          # pseudo-points per chunk (= matmul contraction dim)
WIN = 12         # max distinct segments per chunk (= one-hot width)
CPW = 42         # chunks per window (42*12 = 504 <= 512 fp32 PSUM bank)
PAD = P - C_OUT  # lhsT column padding to 128 cols so FWL triggers
WB = CPW * C_OUT  # feature elems per window block per partition (3360)
# window type pattern: 1 A window (int8 pairs) : 2 D windows (fp8 e3m4)
APAT = 3         # pattern period; w % APAT == 0 -> A window
F8MAX = 14.0     # fp8 e3m4 scaling target (max finite 15.5, margin)


def _wtype(w):
    # A window last in each period so the program starts on D windows
    # (no pair-add dependency -> matmuls start right after the first DMA)
    return 'A' if w % APAT == APAT - 1 else 'D'


def _frustum():
    iH, iW = IMAGE_SIZE
    fH, fW = FEATURE_SIZE
    ds = np.arange(DBOUND[0], DBOUND[1], DBOUND[2], dtype=np.float32)
    xs = np.linspace(0.0, iW - 1.0, fW, dtype=np.float32)
    ys = np.linspace(0.0, iH - 1.0, fH, dtype=np.float32)
    return np.stack(np.broadcast_arrays(
        xs[None, None, :], ys[None, :, None], ds[:, None, None]), -1
    ).astype(np.float32)  # [D, fH, fW, 3]


def _segments(camera_intrinsics, camera2lidar, img_aug_matrix, lidar_aug_matrix):
    """Replicates reference get_geometry + voxelization in numpy float32.
    Returns (seg[Np] int64, kept[Np] bool)."""
    intr = np.asarray(camera_intrinsics, np.float32)
    c2l = np.asarray(camera2lidar, np.float32)
    img_aug = np.asarray(img_aug_matrix, np.float32)
    lidar_aug = np.asarray(lidar_aug_matrix, np.float32)

    intrins = intr[..., :3, :3]
    post_rots = img_aug[..., :3, :3]
    post_trans = img_aug[..., :3, 3]
    rots = c2l[..., :3, :3]
    trans = c2l[..., :3, 3]
    er = lidar_aug[..., :3, :3]
    et = lidar_aug[..., :3, 3]

    f = _frustum()
    pts = f[None, None] - post_trans[:, :, None, None, None, :]
    ipr = np.linalg.inv(post_rots.astype(np.float64)).astype(np.float32)
    pts = np.einsum('bnij,bndhwj->bndhwi', ipr, pts).astype(np.float32)
    pts = np.concatenate([pts[..., :2] * pts[..., 2:3], pts[..., 2:3]], -1)
    iintr = np.linalg.inv(intrins.astype(np.float64)).astype(np.float32)
    comb = np.einsum('bnij,bnjk->bnik', rots, iintr).astype(np.float32)
    pts = (np.einsum('bnij,bndhwj->bndhwi', comb, pts)
           + trans[:, :, None, None, None, :]).astype(np.float32)
    pts = (np.einsum('bij,bndhwj->bndhwi', er, pts)
           + et[:, None, None, None, None, :]).astype(np.float32)

    Np_ = pts.size // 3
    geom = ((pts - (BX - DX / 2.0)) / DX).astype(np.int32).reshape(Np_, 3)
    kept = ((geom[:, 0] >= 0) & (geom[:, 0] < NX[0])
            & (geom[:, 1] >= 0) & (geom[:, 1] < NX[1])
            & (geom[:, 2] >= 0) & (geom[:, 2] < NX[2]))
    seg = (geom[:, 2].astype(np.int64) * (NX[0] * NX[1])
           + geom[:, 0].astype(np.int64) * NX[1]
           + geom[:, 1].astype(np.int64))
    return seg, kept


def _plan(seg, kept):
    """Sort kept points, shard across cores by original-point count, pack
    into chunks whose pseudo-point granularity depends on the window type
    (A: same-segment pairs, D: singles).

    Returns per-core rows [nchunk,P,2] (-1 pad; D chunks use [...,0] only),
    rel [nchunk,P], slot_seg [nchunk,WIN], plus global counts.
    """
    kidx = np.nonzero(kept)[0].astype(np.int64)
    segk = seg[kidx]
    order = np.argsort(segk, kind='stable')
    rows_sorted = kidx[order]
    seg_sorted = segk[order]
    counts = np.bincount(seg_sorted, minlength=NSEG).astype(np.float32)

    rs = np.flatnonzero(np.r_[True, np.diff(seg_sorted) != 0]).astype(np.int64)
    rlen = np.diff(np.r_[rs, len(seg_sorted)]).astype(np.int64)
    run_seg = seg_sorted[rs]
    npts = len(seg_sorted)
    bounds = [int(round(npts * k / NCORES)) for k in range(NCORES + 1)]

    # first pass per core: build pieces (run, take_pts, poff, slot, chunk,
    # fill0) walking original points with window-type-dependent pairing
    cores_pieces = []
    ri = 0
    pcum = np.concatenate([[0], np.cumsum(rlen)])
    for k in range(NCORES):
        lo, hi = bounds[k], bounds[k + 1]
        while ri + 1 < len(pcum) and pcum[ri + 1] <= lo:
            ri += 1
        pieces = []
        chunk = 0
        fill = 0   # pseudo slots used in current chunk
        d = 0      # distinct segs in current chunk
        p = lo
        rj = ri
        while p < hi:
            run_end = pcum[rj + 1]
            rem = int(min(run_end, hi) - p)
            poff = int(p - pcum[rj])
            while rem:
                g = 2 if _wtype(chunk // CPW) == 'A' else 1
                if fill == P:
                    chunk += 1
                    fill = 0
                    d = 0
                    continue
                if d == WIN:  # chunk out of slots: pad to chunk end
                    chunk += 1
                    fill = 0
                    d = 0
                    continue
                slot = d
                d += 1
                take = min(rem, (P - fill) * g)
                npse = (take + g - 1) // g
                pieces.append((rj, take, poff, slot, chunk, fill, g))
                fill += npse
                rem -= take
                poff += take
            p = int(min(run_end, hi))
            if p >= run_end:
                rj += 1
        cores_pieces.append((pieces, chunk + (1 if fill else 0)))

    nchunk = max(nc_ for _, nc_ in cores_pieces)
    nchunk = max(CPW, ((nchunk + CPW - 1) // CPW) * CPW)

    out = []
    for k, (pieces, _) in enumerate(cores_pieces):
        rows = np.full((nchunk, P, 2), -1, np.int64)
        rel = np.full((nchunk, P), -1, np.int32)
        slot_seg = np.full((nchunk, WIN), -1, np.int64)
        for (rj, take, poff, slot, c, fill0, g) in pieces:
            npse = (take + g - 1) // g
            vals = rows_sorted[rs[rj] + poff: rs[rj] + poff + take]
            arr = np.full(npse * g, -1, np.int64)
            arr[:take] = vals
            rows[c, fill0:fill0 + npse, :g] = arr.reshape(npse, g)
            rel[c, fill0:fill0 + npse] = slot
            slot_seg[c, slot] = run_seg[rj]
        out.append(dict(rows=rows, rel=rel, slot_seg=slot_seg))
    return dict(nchunk=nchunk, cores=out, counts=counts)


# ---------------- device program ----------------
_COMPILED = {}


def _build_program(nchunk):
    import concourse.tile as tile
    from concourse import bacc, mybir
    import concourse.bass as bass

    key = (nchunk, WIN, APAT)
    if key in _COMPILED:
        return _COMPILED[key]

    nwin = nchunk // CPW
    nwinA = len([w for w in range(nwin) if _wtype(w) == 'A'])
    nwinD = nwin - nwinA
    bf = mybir.dt.bfloat16
    i8 = mybir.dt.int8
    nc = bacc.Bacc("TRN2", target_bir_lowering=False, debug=False,
                   enable_asserts=False, num_devices=NCORES)
    f8 = mybir.dt.float8e3
    ptsa = nc.dram_tensor("ptsa", [P, max(1, nwinA * 2 * WB)], i8,
                          kind="ExternalInput").ap()
    ptsd = nc.dram_tensor("ptsd", [P, max(1, nwinD * WB)], f8,
                          kind="ExternalInput").ap()
    rel = nc.dram_tensor("rel", [P, nchunk], bf, kind="ExternalInput").ap()
    iota = nc.dram_tensor("iota", [P, WIN], bf, kind="ExternalInput").ap()
    wout = nc.dram_tensor("wout", [nwin, C_OUT, CPW * WIN], bf,
                          kind="ExternalOutput").ap()

    with tile.TileContext(nc) as tc:
        with tc.tile_pool(name="const", bufs=1) as constp, \
             tc.tile_pool(name="feata", bufs=6) as featap, \
             tc.tile_pool(name="featd", bufs=6) as featdp, \
             tc.tile_pool(name="pair", bufs=6) as pairp, \
             tc.tile_pool(name="oh", bufs=8) as ohp, \
             tc.tile_pool(name="stage", bufs=8) as stagep, \
             tc.tile_pool(name="psum", bufs=8, space="PSUM") as psump:
            rel_t = constp.tile([P, nchunk], bf)
            nc.sync.dma_start(out=rel_t[:], in_=rel[:])
            iota_t = constp.tile([P, WIN], bf)
            nc.sync.dma_start(out=iota_t[:], in_=iota[:])

            a_i = 0
            d_i = 0
            for w0 in range(0, nwin, APAT):
                ws = range(w0, min(w0 + APAT, nwin))
                # input DMAs for the whole period first, split across queues
                fsrc = {}
                dfirst = True
                for w in ws:
                    if _wtype(w) == 'A':
                        f_t = featap.tile([P, 2 * WB], i8)
                        nc.sync.dma_start(
                            out=f_t[:],
                            in_=ptsa[:, a_i * 2 * WB:(a_i + 1) * 2 * WB])
                        a_i += 1
                        fsrc[w] = (f_t, 0)
                    else:
                        dt = featdp.tile([P, WB + PAD], f8)
                        eng = nc.scalar if dfirst else nc.sync
                        dfirst = False
                        eng.dma_start(
                            out=dt[:, 0:WB],
                            in_=ptsd[:, d_i * WB:(d_i + 1) * WB])
                        d_i += 1
                        fsrc[w] = (dt, 0)
                # one-hots for the period (DVE; only need rel_t)
                ohs = {}
                for w in ws:
                    oh = ohp.tile([P, CPW, WIN],
                                  bf if _wtype(w) == 'A' else f8)
                    rsl = rel_t[:, w * CPW:(w + 1) * CPW]
                    rel_b = bass.AP(rsl.tensor, rsl.offset,
                                    list(rsl.ap) + [[0, WIN]])
                    iap = iota_t[:]
                    iota_b = bass.AP(iap.tensor, iap.offset,
                                     [iap.ap[0], [0, CPW], iap.ap[1]])
                    nc.vector.tensor_tensor(out=oh[:], in0=iota_b, in1=rel_b,
                                            op=mybir.AluOpType.is_equal)
                    ohs[w] = oh
                # compute per window
                for w in ws:
                    tsrc, off = fsrc[w]
                    if _wtype(w) == 'A':
                        src = pairp.tile([P, WB + PAD], bf)
                        hb = (CPW // 2) * C_OUT
                        nc.vector.tensor_tensor(
                            out=src[:, 0:hb], in0=tsrc[:, 0:hb],
                            in1=tsrc[:, WB:WB + hb], op=mybir.AluOpType.add)
                        nc.vector.tensor_tensor(
                            out=src[:, hb:WB], in0=tsrc[:, hb:WB],
                            in1=tsrc[:, WB + hb:2 * WB],
                            op=mybir.AluOpType.add)
                        off = 0
                    else:
                        src = tsrc
                    oh = ohs[w]
                    ps = psump.tile([P, 512], mybir.dt.float32)
                    for c in range(CPW):
                        # stationary = features padded to 128 cols (junk-read)
                        nc.tensor.matmul(
                            out=ps[0:P, c * WIN:(c + 1) * WIN],
                            lhsT=src[:, off + c * C_OUT:off + c * C_OUT + P],
                            rhs=oh[:, c],
                            start=True,
                            stop=True,
                        )
                    st = stagep.tile([C_OUT, CPW * WIN], bf)
                    nc.scalar.copy(out=st[:], in_=ps[0:C_OUT, 0:CPW * WIN])
                    nc.scalar.dma_start(out=wout[w], in_=st[:])

    nc.compile()
    _COMPILED[key] = nc
    return nc


def _run_on_hw(nc, in_maps, trace=False):
    from concourse.bass_utils import run_bass_kernel_spmd
    from concourse.bass_interp import get_hw_module

    if trace:
        try:
            import ntff_hook
            ntff_hook.install()
        except Exception:
            pass
    hw_m = get_hw_module(nc.m)
    old_m = nc.m
    nc.m = hw_m
    try:
        res = run_bass_kernel_spmd(
            nc, in_maps, core_ids=list(range(NCORES)), trace=trace,
        )
    finally:
        nc.m = old_m
    return res


def kernel(cam_feats, camera_intrinsics, camera2lidar, img_aug_matrix,
           lidar_aug_matrix, _trace=False, _return_results=False):
    cam = np.ascontiguousarray(np.asarray(cam_feats, np.float32))
    Npts = cam.size // C_OUT
    cam_f = cam.reshape(Npts, C_OUT)
    cam_aug = np.vstack([cam_f, np.zeros((1, C_OUT), np.float32)])

    seg, kept = _segments(camera_intrinsics, camera2lidar,
                          img_aug_matrix, lidar_aug_matrix)
    plan = _plan(seg, kept)
    nchunk = plan['nchunk']
    nwin = nchunk // CPW
    wtypes = np.array([_wtype(w) == 'A' for w in range(nwin)])
    a_chunks = np.repeat(wtypes, CPW)

    iota_c = np.broadcast_to(np.arange(WIN, dtype=np.float32),
                             (P, WIN)).astype(ml_dtypes.bfloat16)
    in_maps = []
    scales = []
    for k in range(NCORES):
        ck = plan['cores'][k]
        s = np.ones(nchunk, np.float32)
        # A windows: int8 pairs
        rows_a = ck['rows'][a_chunks]
        fa = cam_aug[rows_a.reshape(-1)].reshape(-1, P, 2, C_OUT)
        aa = np.abs(fa).reshape(len(fa), -1).max(axis=1)
        sa = np.where(aa > 0, aa / 127.0, 1.0).astype(np.float32)
        qa = np.clip(np.rint(fa / sa[:, None, None, None]),
                     -127, 127).astype(np.int8)
        s[a_chunks] = sa
        nwa = len(fa) // CPW
        pa = qa.reshape(nwa, CPW, P, 2, C_OUT).transpose(2, 0, 3, 1, 4)
        pa = np.ascontiguousarray(pa).reshape(P, nwa * 2 * CPW * C_OUT)
        # D windows: fp8 e3m4 direct with per-chunk scale
        rows_d = ck['rows'][~a_chunks][:, :, 0]
        fd = cam_aug[rows_d.reshape(-1)].reshape(-1, P, C_OUT)
        ad = np.abs(fd).reshape(len(fd), -1).max(axis=1)
        sd = np.where(ad > 0, ad / F8MAX, 1.0).astype(np.float32)
        qd = (fd / sd[:, None, None]).astype(ml_dtypes.float8_e3m4)
        s[~a_chunks] = sd
        nwd = len(fd) // CPW
        pd = qd.reshape(nwd, CPW, P, C_OUT).transpose(2, 0, 1, 3)
        pd = np.ascontiguousarray(pd).reshape(P, max(1, nwd * CPW * C_OUT))
        scales.append(s)
        relk = np.ascontiguousarray(
            ck['rel'].T.astype(np.float32)).astype(ml_dtypes.bfloat16)
        in_maps.append(dict(ptsa=pa, ptsd=pd, rel=relk, iota=iota_c))

    nc = _build_program(nchunk)
    res = _run_on_hw(nc, in_maps, trace=_trace)

    # ---------------- host assembly ----------------
    s_parts = []
    v_parts = []
    for k in range(NCORES):
        vals = np.asarray(res.results[k]['wout']).astype(np.float32)
        vals = vals.reshape(nwin, C_OUT, CPW, WIN).transpose(0, 2, 3, 1)
        vals = vals.reshape(nchunk, WIN, C_OUT) * scales[k][:, None, None]
        cseg = plan['cores'][k]['slot_seg']
        m = cseg >= 0
        s_parts.append(cseg[m])
        v_parts.append(vals[m])
    s_all = np.concatenate(s_parts)
    v_all = np.concatenate(v_parts)
    acc = np.zeros((NSEG, C_OUT), np.float32)
    if len(s_all):
        o2 = np.argsort(s_all, kind='stable')
        s2 = s_all[o2]
        v2 = v_all[o2]
        starts = np.r_[0, np.flatnonzero(np.diff(s2)) + 1]
        sums = np.add.reduceat(v2, starts, axis=0)
        useg = s2[starts]
        acc[useg] = sums / np.maximum(plan['counts'][useg], 1.0)[:, None]

    out = acc.reshape(NX[2], NX[0], NX[1], C_OUT).transpose(0, 3, 1, 2)
    out = out.reshape(1, NX[2] * C_OUT, NX[0], NX[1]).astype(np.float32)
    if _return_results:
        return out, res
    return out


# revision 45
# speedup vs baseline: 1.1987x; 1.0121x over previous
"""Trainium2 Bass kernel for nn_BaseViewTransform (BEVFusion bev_pool / segment-mean).

Pipeline (v3 — hybrid int8-pair / bf16-direct windows):
  The machine balance: HBM DMA wants few bytes (int8), but int8 needs an
  on-device convert to bf16 for the PE, and only the DVE can do that at
  ~1 elem/ns/partition (GPSIMD/multi-engine runs CONTEND and go slower).
  So windows alternate between two types:
    A (int8 pairs):  80 B/pt DMA + DVE pair-add (int8+int8 -> bf16, exact)
    D (bf16 direct): 160 B/pt DMA + zero DVE work (feats feed the PE
                     straight from the DMA tile)
  which equalizes the DMA wall and the DVE wall.

  Host (index plane only): compute per-point voxel ids exactly as the
  reference, sort kept points by segment, shard across 8 cores, pack into
  128-pseudo chunks (pseudo = same-segment pair in A windows, single point
  in D windows) with <= WIN=12 distinct segments per chunk; per-chunk int8
  scales for A chunks.
  Device, per 42-chunk window: DMA stream in; [A only] DVE pair-add;
  DVE one-hot via is_equal; 42 matmuls with the FEATURES as the stationary
  operand padded to 128 columns (FWL) and the one-hot moving (12 cols);
  PSUM [0:80] -> SBUF bf16 via ACT; DMA out.
  Host: scale chunk sums, reduce per segment, divide by counts, scatter.
"""

import numpy as np
import ml_dtypes

# ---------------- problem constants (hardcoded per task rules) ----------------
IMAGE_SIZE = (256, 704)
FEATURE_SIZE = (32, 88)
XBOUND = (-54.0, 54.0, 0.3)
YBOUND = (-54.0, 54.0, 0.3)
ZBOUND = (-10.0, 10.0, 20.0)
DBOUND = (1.0, 60.0, 0.5)
C_OUT = 80
NX = (360, 360, 1)
NSEG = NX[2] * NX[0] * NX[1]  # 129600
DX = np.array([XBOUND[2], YBOUND[2], ZBOUND[2]], np.float32)
BX = np.array([XBOUND[0] + XBOUND[2] / 2.0,
               YBOUND[0] + YBOUND[2] / 2.0,
               ZBOUND[0] + ZBOUND[2] / 2.0], np.float32)

NCORES = 8
P = 128          # pseudo-points per chunk (= matmul contraction dim)
WIN = 12         # max distinct segments per chunk (= one-hot width)
CPW = 42         # chunks per window (42*12 = 504 <= 512 fp32 PSUM bank)
PAD = P - C_OUT  # lhsT column padding to 128 cols so FWL triggers
WB = CPW * C_OUT  # feature elems per window block per partition (3360)
# window type pattern: 1 A window (int8 pairs) : 2 D windows (fp8 e3m4)
APAT = 3         # pattern period; w % APAT == 0 -> A window
F8MAX = 14.0     # fp8 e3m4 scaling target (max finite 15.5, margin)


def _wtype(w):
    # A window last in each period so the program starts on D windows
    # (no pair-add dependency -> matmuls start right after the first DMA)
    return 'A' if w % APAT == APAT - 1 else 'D'


def _frustum():
    iH, iW = IMAGE_SIZE
    fH, fW = FEATURE_SIZE
    ds = np.arange(DBOUND[0], DBOUND[1], DBOUND[2], dtype=np.float32)
    xs = np.linspace(0.0, iW - 1.0, fW, dtype=np.float32)
    ys = np.linspace(0.0, iH - 1.0, fH, dtype=np.float32)
    return np.stack(np.broadcast_arrays(
        xs[None, None, :], ys[None, :, None], ds[:, None, None]), -1
    ).astype(np.float32)  # [D, fH, fW, 3]


def _segments(camera_intrinsics, camera2lidar, img_aug_matrix, lidar_aug_matrix):
    """Replicates reference get_geometry + voxelization in numpy float32.
    Returns (seg[Np] int64, kept[Np] bool)."""
    intr = np.asarray(camera_intrinsics, np.float32)
    c2l = np.asarray(camera2lidar, np.float32)
    img_aug = np.asarray(img_aug_matrix, np.float32)
    lidar_aug = np.asarray(lidar_aug_matrix, np.float32)

    intrins = intr[..., :3, :3]
    post_rots = img_aug[..., :3, :3]
    post_trans = img_aug[..., :3, 3]
    rots = c2l[..., :3, :3]
    trans = c2l[..., :3, 3]
    er = lidar_aug[..., :3, :3]
    et = lidar_aug[..., :3, 3]

    f = _frustum()
    pts = f[None, None] - post_trans[:, :, None, None, None, :]
    ipr = np.linalg.inv(post_rots.astype(np.float64)).astype(np.float32)
    pts = np.einsum('bnij,bndhwj->bndhwi', ipr, pts).astype(np.float32)
    pts = np.concatenate([pts[..., :2] * pts[..., 2:3], pts[..., 2:3]], -1)
    iintr = np.linalg.inv(intrins.astype(np.float64)).astype(np.float32)
    comb = np.einsum('bnij,bnjk->bnik', rots, iintr).astype(np.float32)
    pts = (np.einsum('bnij,bndhwj->bndhwi', comb, pts)
           + trans[:, :, None, None, None, :]).astype(np.float32)
    pts = (np.einsum('bij,bndhwj->bndhwi', er, pts)
           + et[:, None, None, None, None, :]).astype(np.float32)

    Np_ = pts.size // 3
    geom = ((pts - (BX - DX / 2.0)) / DX).astype(np.int32).reshape(Np_, 3)
    kept = ((geom[:, 0] >= 0) & (geom[:, 0] < NX[0])
            & (geom[:, 1] >= 0) & (geom[:, 1] < NX[1])
            & (geom[:, 2] >= 0) & (geom[:, 2] < NX[2]))
    seg = (geom[:, 2].astype(np.int64) * (NX[0] * NX[1])
           + geom[:, 0].astype(np.int64) * NX[1]
           + geom[:, 1].astype(np.int64))
    return seg, kept


def _plan(seg, kept):
    """Sort kept points, shard across cores by original-point count, pack
    into chunks whose pseudo-point granularity depends on the window type
    (A: same-segment pairs, D: singles).

    Returns per-core rows [nchunk,P,2] (-1 pad; D chunks use [...,0] only),
    rel [nchunk,P], slot_seg [nchunk,WIN], plus global counts.
    """
    kidx = np.nonzero(kept)[0].astype(np.int64)
    segk = seg[kidx]
    order = np.argsort(segk, kind='stable')
    rows_sorted = kidx[order]
    seg_sorted = segk[order]
    counts = np.bincount(seg_sorted, minlength=NSEG).astype(np.float32)

    rs = np.flatnonzero(np.r_[True, np.diff(seg_sorted) != 0]).astype(np.int64)
    rlen = np.diff(np.r_[rs, len(seg_sorted)]).astype(np.int64)
    run_seg = seg_sorted[rs]
    npts = len(seg_sorted)
    bounds = [int(round(npts * k / NCORES)) for k in range(NCORES + 1)]

    # first pass per core: build pieces (run, take_pts, poff, slot, chunk,
    # fill0) walking original points with window-type-dependent pairing
    cores_pieces = []
    ri = 0
    pcum = np.concatenate([[0], np.cumsum(rlen)])
    for k in range(NCORES):
        lo, hi = bounds[k], bounds[k + 1]
        while ri + 1 < len(pcum) and pcum[ri + 1] <= lo:
            ri += 1
        pieces = []
        chunk = 0
        fill = 0   # pseudo slots used in current chunk
        d = 0      # distinct segs in current chunk
        p = lo
        rj = ri
        while p < hi:
            run_end = pcum[rj + 1]
            rem = int(min(run_end, hi) - p)
            poff = int(p - pcum[rj])
            while rem:
                g = 2 if _wtype(chunk // CPW) == 'A' else 1
                if fill == P:
                    chunk += 1
                    fill = 0
                    d = 0
                    continue
                if d == WIN:  # chunk out of slots: pad to chunk end
                    chunk += 1
                    fill = 0
                    d = 0
                    continue
                slot = d
                d += 1
                take = min(rem, (P - fill) * g)
                npse = (take + g - 1) // g
                pieces.append((rj, take, poff, slot, chunk, fill, g))
                fill += npse
                rem -= take
                poff += take
            p = int(min(run_end, hi))
            if p >= run_end:
                rj += 1
        cores_pieces.append((pieces, chunk + (1 if fill else 0)))

    nchunk = max(nc_ for _, nc_ in cores_pieces)
    nchunk = max(CPW, ((nchunk + CPW - 1) // CPW) * CPW)

    out = []
    for k, (pieces, _) in enumerate(cores_pieces):
        rows = np.full((nchunk, P, 2), -1, np.int64)
        rel = np.full((nchunk, P), -1, np.int32)
        slot_seg = np.full((nchunk, WIN), -1, np.int64)
        for (rj, take, poff, slot, c, fill0, g) in pieces:
            npse = (take + g - 1) // g
            vals = rows_sorted[rs[rj] + poff: rs[rj] + poff + take]
            arr = np.full(npse * g, -1, np.int64)
            arr[:take] = vals
            rows[c, fill0:fill0 + npse, :g] = arr.reshape(npse, g)
            rel[c, fill0:fill0 + npse] = slot
            slot_seg[c, slot] = run_seg[rj]
        out.append(dict(rows=rows, rel=rel, slot_seg=slot_seg))
    return dict(nchunk=nchunk, cores=out, counts=counts)


# ---------------- device program ----------------
_COMPILED = {}


def _build_program(nchunk):
    import concourse.tile as tile
    from concourse import bacc, mybir
    import concourse.bass as bass

    key = (nchunk, WIN, APAT)
    if key in _COMPILED:
        return _COMPILED[key]

    nwin = nchunk // CPW
    nwinA = len([w for w in range(nwin) if _wtype(w) == 'A'])
    nwinD = nwin - nwinA
    bf = mybir.dt.bfloat16
    i8 = mybir.dt.int8
    nc = bacc.Bacc("TRN2", target_bir_lowering=False, debug=False,
                   enable_asserts=False, num_devices=NCORES)
    f8 = mybir.dt.float8e3
    ptsa = nc.dram_tensor("ptsa", [P, max(1, nwinA * 2 * WB)], i8,
                          kind="ExternalInput").ap()
    ptsd = nc.dram_tensor("ptsd", [P, max(1, nwinD * WB)], f8,
                          kind="ExternalInput").ap()
    rel = nc.dram_tensor("rel", [P, nchunk], i8, kind="ExternalInput").ap()
    iota = nc.dram_tensor("iota", [P, WIN], i8, kind="ExternalInput").ap()
    wout = nc.dram_tensor("wout", [nwin, C_OUT, CPW * WIN], bf,
                          kind="ExternalOutput").ap()

    with tile.TileContext(nc) as tc:
        with tc.tile_pool(name="const", bufs=1) as constp, \
             tc.tile_pool(name="feata", bufs=6) as featap, \
             tc.tile_pool(name="featd", bufs=6) as featdp, \
             tc.tile_pool(name="pair", bufs=6) as pairp, \
             tc.tile_pool(name="oh", bufs=8) as ohp, \
             tc.tile_pool(name="stage", bufs=8) as stagep, \
             tc.tile_pool(name="psum", bufs=8, space="PSUM") as psump:
            rel_t = constp.tile([P, nchunk], i8)
            nc.sync.dma_start(out=rel_t[:], in_=rel[:])
            iota_t = constp.tile([P, WIN], i8)
            nc.sync.dma_start(out=iota_t[:], in_=iota[:])

            a_i = 0
            d_i = 0
            for w0 in range(0, nwin, APAT):
                ws = range(w0, min(w0 + APAT, nwin))
                # input DMAs for the whole period first, split across queues
                fsrc = {}
                dfirst = True
                for w in ws:
                    if _wtype(w) == 'A':
                        f_t = featap.tile([P, 2 * WB], i8)
                        nc.sync.dma_start(
                            out=f_t[:],
                            in_=ptsa[:, a_i * 2 * WB:(a_i + 1) * 2 * WB])
                        a_i += 1
                        fsrc[w] = (f_t, 0)
                    else:
                        dt = featdp.tile([P, WB + PAD], f8)
                        eng = nc.scalar if dfirst else nc.sync
                        dfirst = False
                        eng.dma_start(
                            out=dt[:, 0:WB],
                            in_=ptsd[:, d_i * WB:(d_i + 1) * WB])
                        d_i += 1
                        fsrc[w] = (dt, 0)
                # one-hots for the period (DVE; only need rel_t)
                ohs = {}
                for w in ws:
                    oh = ohp.tile([P, CPW, WIN],
                                  bf if _wtype(w) == 'A' else f8)
                    rsl = rel_t[:, w * CPW:(w + 1) * CPW]
                    rel_b = bass.AP(rsl.tensor, rsl.offset,
                                    list(rsl.ap) + [[0, WIN]])
                    iap = iota_t[:]
                    iota_b = bass.AP(iap.tensor, iap.offset,
                                     [iap.ap[0], [0, CPW], iap.ap[1]])
                    nc.vector.tensor_tensor(out=oh[:], in0=iota_b, in1=rel_b,
                                            op=mybir.AluOpType.is_equal)
                    ohs[w] = oh
                # compute per window
                for w in ws:
                    tsrc, off = fsrc[w]
                    if _wtype(w) == 'A':
                        src = pairp.tile([P, WB + PAD], bf)
                        hb = (CPW // 2) * C_OUT
                        nc.vector.tensor_tensor(
                            out=src[:, 0:hb], in0=tsrc[:, 0:hb],
                            in1=tsrc[:, WB:WB + hb], op=mybir.AluOpType.add)
                        nc.vector.tensor_tensor(
                            out=src[:, hb:WB], in0=tsrc[:, hb:WB],
                            in1=tsrc[:, WB + hb:2 * WB],
                            op=mybir.AluOpType.add)
                        off = 0
                    else:
                        src = tsrc
                    oh = ohs[w]
                    ps = psump.tile([P, 512], mybir.dt.float32)
                    for c in range(CPW):
                        # stationary = features padded to 128 cols (junk-read)
                        nc.tensor.matmul(
                            out=ps[0:P, c * WIN:(c + 1) * WIN],
                            lhsT=src[:, off + c * C_OUT:off + c * C_OUT + P],
                            rhs=oh[:, c],
                            start=True,
                            stop=True,
                        )
                    st = stagep.tile([C_OUT, CPW * WIN], bf)
                    nc.scalar.copy(out=st[:], in_=ps[0:C_OUT, 0:CPW * WIN])
                    nc.scalar.dma_start(out=wout[w], in_=st[:])

    nc.compile()
    _COMPILED[key] = nc
    return nc


def _run_on_hw(nc, in_maps, trace=False):
    from concourse.bass_utils import run_bass_kernel_spmd
    from concourse.bass_interp import get_hw_module

    if trace:
        try:
            import ntff_hook
            ntff_hook.install()
        except Exception:
            pass
    hw_m = get_hw_module(nc.m)
    old_m = nc.m
    nc.m = hw_m
    try:
        res = run_bass_kernel_spmd(
            nc, in_maps, core_ids=list(range(NCORES)), trace=trace,
        )
    finally:
        nc.m = old_m
    return res


def kernel(cam_feats, camera_intrinsics, camera2lidar, img_aug_matrix,
           lidar_aug_matrix, _trace=False, _return_results=False):
    cam = np.ascontiguousarray(np.asarray(cam_feats, np.float32))
    Npts = cam.size // C_OUT
    cam_f = cam.reshape(Npts, C_OUT)
    cam_aug = np.vstack([cam_f, np.zeros((1, C_OUT), np.float32)])

    seg, kept = _segments(camera_intrinsics, camera2lidar,
                          img_aug_matrix, lidar_aug_matrix)
    plan = _plan(seg, kept)
    nchunk = plan['nchunk']
    nwin = nchunk // CPW
    wtypes = np.array([_wtype(w) == 'A' for w in range(nwin)])
    a_chunks = np.repeat(wtypes, CPW)

    iota_c = np.ascontiguousarray(
        np.broadcast_to(np.arange(WIN, dtype=np.int8), (P, WIN)))
    in_maps = []
    scales = []
    for k in range(NCORES):
        ck = plan['cores'][k]
        s = np.ones(nchunk, np.float32)
        # A windows: int8 pairs
        rows_a = ck['rows'][a_chunks]
        fa = cam_aug[rows_a.reshape(-1)].reshape(-1, P, 2, C_OUT)
        aa = np.abs(fa).reshape(len(fa), -1).max(axis=1)
        sa = np.where(aa > 0, aa / 127.0, 1.0).astype(np.float32)
        qa = np.clip(np.rint(fa / sa[:, None, None, None]),
                     -127, 127).astype(np.int8)
        s[a_chunks] = sa
        nwa = len(fa) // CPW
        pa = qa.reshape(nwa, CPW, P, 2, C_OUT).transpose(2, 0, 3, 1, 4)
        pa = np.ascontiguousarray(pa).reshape(P, nwa * 2 * CPW * C_OUT)
        # D windows: fp8 e3m4 direct with per-chunk scale
        rows_d = ck['rows'][~a_chunks][:, :, 0]
        fd = cam_aug[rows_d.reshape(-1)].reshape(-1, P, C_OUT)
        ad = np.abs(fd).reshape(len(fd), -1).max(axis=1)
        sd = np.where(ad > 0, ad / F8MAX, 1.0).astype(np.float32)
        qd = (fd / sd[:, None, None]).astype(ml_dtypes.float8_e3m4)
        s[~a_chunks] = sd
        nwd = len(fd) // CPW
        pd = qd.reshape(nwd, CPW, P, C_OUT).transpose(2, 0, 1, 3)
        pd = np.ascontiguousarray(pd).reshape(P, max(1, nwd * CPW * C_OUT))
        scales.append(s)
        relk = np.ascontiguousarray(ck['rel'].T.astype(np.int8))
        in_maps.append(dict(ptsa=pa, ptsd=pd, rel=relk, iota=iota_c))

    nc = _build_program(nchunk)
    res = _run_on_hw(nc, in_maps, trace=_trace)

    # ---------------- host assembly ----------------
    s_parts = []
    v_parts = []
    for k in range(NCORES):
        vals = np.asarray(res.results[k]['wout']).astype(np.float32)
        vals = vals.reshape(nwin, C_OUT, CPW, WIN).transpose(0, 2, 3, 1)
        vals = vals.reshape(nchunk, WIN, C_OUT) * scales[k][:, None, None]
        cseg = plan['cores'][k]['slot_seg']
        m = cseg >= 0
        s_parts.append(cseg[m])
        v_parts.append(vals[m])
    s_all = np.concatenate(s_parts)
    v_all = np.concatenate(v_parts)
    acc = np.zeros((NSEG, C_OUT), np.float32)
    if len(s_all):
        o2 = np.argsort(s_all, kind='stable')
        s2 = s_all[o2]
        v2 = v_all[o2]
        starts = np.r_[0, np.flatnonzero(np.diff(s2)) + 1]
        sums = np.add.reduceat(v2, starts, axis=0)
        useg = s2[starts]
        acc[useg] = sums / np.maximum(plan['counts'][useg], 1.0)[:, None]

    out = acc.reshape(NX[2], NX[0], NX[1], C_OUT).transpose(0, 3, 1, 2)
    out = out.reshape(1, NX[2] * C_OUT, NX[0], NX[1]).astype(np.float32)
    if _return_results:
        return out, res
    return out
